# revision 2
# baseline (speedup 1.0000x reference)
"""CAREConv forward kernel v2 for Trainium2 (8 NeuronCores, Bass/Tile).

Math (per node i with D=32 in-edges grouped by destination):
    t = tanh(feature @ W_mlp.T + b_mlp)            # [N, 2]
    d[i, j] = |t[src[i,j]] - t[i]|.sum()           # L1 dist, [N, D]
    keep K=16 smallest-d in-edges (ties -> lower j, matching lax.top_k)
    h_et[i] = mean_k feature[src[i, keep_k]]       # [N, F]
    out = (0.5 * h_et + feature) @ W_lin.T + b_lin # [N, H]

v2 replaces the baseline's ~4700 one-offset-per-partition indirect DMAs
(~1.1us each, SWDGE-serial => 5.4ms) with batched InstDMAGatherAnt:

  * per-edge t: one dma_gather per dst tile over a 256B-row window table
    twin[w] = t[4w .. 4w+31] (f32 pairs).  idx = src//4 < 25088 fits int16
    in a single sweep; q = src & 3 resolved on-chip with 4 mask-mul-adds.
  * selected features: bf16 table in 4 chunks of 32000 rows, each with a
    leading zero row.  Per 4-tile batch, 4 transpose-mode gathers (one per
    chunk) with idx = in-chunk ? local+1 : 0; out-of-chunk slots read the
    zero row, so summing the 4 sweep buffers composes exactly.  Transpose
    mode lands features F-major ([128 F, col], col = tile*2048+p*16+k),
    feeding the tree-sum, residual and final W_lin matmul with no PE
    transpose.

Selection keeps the baseline max8/match_replace machinery (exact
lax.top_k tie semantics).
"""

import numpy as np
import ml_dtypes

import concourse.bacc as bacc
import concourse.bass as bass
import concourse.tile as tile
from concourse import mybir
from concourse.bass_utils import run_bass_kernel_spmd
from concourse.masks import make_identity
from concourse.tile import add_dep_helper

F32 = mybir.dt.float32
I32 = mybir.dt.int32
I16 = mybir.dt.int16
BF16 = mybir.dt.bfloat16

N = 100_000      # real nodes
D = 32           # in-degree
K = 16           # neighbors kept (ceil(D * 0.5))
F = 128          # IN_FEATS
H = 64           # H_FEATS
C = 2            # NUM_CLASSES (t width)
PKEEP = 0.5
NCORES = 8
P = 128
SHARD = 12_544
NPAD = SHARD * NCORES  # 100352
TILES = SHARD // P     # 98

MINVAL = float(-(2 ** 30))

CH = 32_000              # nodes per feature chunk
CHROWS = CH + 1          # +1 leading zero row
NCHUNK = 4
WROWS = NPAD // 4        # 25088 t-window rows
TLOCPAD = NPAD + 32      # t_loc rows incl. window overhang

TB = 4                   # dst tiles per phase-2 batch
NBATCH = TILES // TB     # 24 full batches (+1 tail of 2)
NTI = P * D              # t-gather idxs per tile = 4096


def build():
    nc = bacc.Bacc("TRN2", target_bir_lowering=False, debug=False,
                   num_devices=NCORES)

    feat_own = nc.dram_tensor("feat_own", [SHARD, F], F32,
                              kind="ExternalInput")
    fbts = [nc.dram_tensor(f"fbt{c}", [CHROWS, F], BF16,
                           kind="ExternalInput") for c in range(NCHUNK)]
    src_own = nc.dram_tensor("src_own", [SHARD, D], I32,
                             kind="ExternalInput")
    tg_idx = nc.dram_tensor("tg_idx", [P, TILES * (NTI // 16)], I16,
                            kind="ExternalInput")
    rep_mat = nc.dram_tensor("rep_mat", [K, P], F32, kind="ExternalInput")
    w_mlp_t = nc.dram_tensor("w_mlp_t", [F, C], F32, kind="ExternalInput")
    b_mlp = nc.dram_tensor("b_mlp", [C, 1], F32, kind="ExternalInput")
    w_lin_t = nc.dram_tensor("w_lin_t", [F, H], F32, kind="ExternalInput")
    w_lin_s = nc.dram_tensor("w_lin_s", [F, H], F32, kind="ExternalInput")
    b_lin = nc.dram_tensor("b_lin", [H, 1], F32, kind="ExternalInput")
    out_t = nc.dram_tensor("out_t", [H, SHARD], F32, kind="ExternalOutput")

    t_loc = nc.dram_tensor("t_loc", [TLOCPAD, C], F32, kind="Internal")
    twin = nc.dram_tensor("twin", [WROWS, 64], F32, kind="Internal")
    base = nc.dram_tensor("base", [H, SHARD], F32, kind="Internal")

    ts = bass.ts

    with tile.TileContext(nc) as tc:
        with (
            tc.tile_pool(name="const", bufs=1) as cpool,
            tc.tile_pool(name="persist", bufs=1) as ppool,
            tc.tile_pool(name="dram", bufs=1, space="DRAM") as dpool,
        ):
            ident = cpool.tile([P, P], F32)
            make_identity(nc, ident[:])
            wm = cpool.tile([F, C], F32)
            nc.sync.dma_start(wm[:], w_mlp_t[:, :])
            wl = cpool.tile([F, H], F32)
            nc.sync.dma_start(wl[:], w_lin_t[:, :])
            wls = cpool.tile([F, H], F32)
            nc.sync.dma_start(wls[:], w_lin_s[:, :])
            bm = cpool.tile([C, 1], F32)
            nc.sync.dma_start(bm[:], b_mlp[:, :])
            bl = cpool.tile([H, 1], F32)
            nc.sync.dma_start(bl[:], b_lin[:, :])
            rep = cpool.tile([K, P], F32)
            nc.sync.dma_start(rep[:], rep_mat[:, :])

            tneg = ppool.tile([P, TILES * C], F32)    # -t_own per dst

            t_shard = dpool.tile([SHARD, C], F32)
            t_full = dpool.tile([NPAD, C], F32, addr_space="Shared")

            # ---------------- Phase 1: t = tanh(feat @ Wmlp.T + b) ----------
            with (
                tc.tile_pool(name="p1", bufs=3) as p1,
                tc.tile_pool(name="p1ps", bufs=2, space="PSUM") as p1ps,
            ):
                zt = p1.tile([32, C], F32, tag="zt")
                nc.vector.memset(zt[:], 0.0)
                ztw = nc.sync.dma_start(t_loc[NPAD:TLOCPAD, :], zt[:])
                base_writes = [ztw]
                for i in range(TILES):
                    ft = p1.tile([P, F], F32, tag="ft")
                    nc.sync.dma_start(ft[:], feat_own[ts(i, P), :])
                    ps_tr = p1ps.tile([P, P], F32, tag="ps_tr")
                    nc.tensor.transpose(ps_tr[:], ft[:], ident[:])
                    fT = p1.tile([P, P], F32, tag="fT")
                    nc.scalar.copy(fT[:], ps_tr[:])
                    ps_z = p1ps.tile([C, P], F32, tag="ps_z")
                    nc.tensor.matmul(out=ps_z[:], lhsT=wm[:], rhs=fT[:],
                                     start=True, stop=True)
                    tk = p1.tile([C, P], F32, tag="tk")
                    nc.scalar.activation(tk[:], ps_z[:],
                                         mybir.ActivationFunctionType.Tanh,
                                         bias=bm[:, 0:1])
                    nc.sync.dma_start(
                        t_shard[ts(i, P), :].rearrange("n c -> c n"), tk[:])
                    ps_to = p1ps.tile([P, C], F32, tag="ps_to")
                    nc.tensor.transpose(ps_to[:], tk[:], ident[:C, :C])
                    nc.scalar.mul(tneg[:, ts(i, C)], ps_to[:], -1.0)
                    # base = feat @ W_lin.T + b_lin (residual term, F-major)
                    ps_b = p1ps.tile([H, P], F32, tag="ps_b")
                    nc.tensor.matmul(out=ps_b[:], lhsT=wl[:], rhs=fT[:],
                                     start=True, stop=True)
                    ob1 = p1.tile([H, P], F32, tag="ob1")
                    nc.vector.tensor_scalar(
                        ob1[:], ps_b[:], bl[:, 0:1], None,
                        op0=mybir.AluOpType.add)
                    bw = nc.sync.dma_start(base[:, ts(i, P)], ob1[:])
                    base_writes.append(bw)

            # ---------------- AllGather t + window table --------------------
            nc.gpsimd.collective_compute(
                "AllGather",
                mybir.AluOpType.bypass,
                replica_groups=[list(range(NCORES))],
                ins=[t_shard[:, :]],
                outs=[t_full[:, :]],
            )
            t_cp = nc.sync.dma_start(t_loc[0:NPAD, :], t_full[:, :])
            # twin[w, m] = t_loc.flat[8w + m]: for each 8-col block j,
            # twin[:, 8j:8j+8] = flat[8j : 8j + 8*WROWS] (contiguous read,
            # strided write) - 8 clean DMAs instead of an overlapping AP.
            tws = []
            for j in range(8):
                twj = nc.sync.dma_start(
                    bass.AP(twin, 8 * j, [[64, WROWS], [1, 8]]),
                    bass.AP(t_loc, 8 * j, [[8, WROWS], [1, 8]]))
                add_dep_helper(twj.ins, t_cp.ins, reason="twin after t copy")
                add_dep_helper(twj.ins, ztw.ins, reason="twin after zero")
                tws.append(twj)

            dr = nc.gpsimd.drain()
            for twj in tws:
                add_dep_helper(dr.ins, twj.ins,
                               reason="drain after twin build")
            for bw in base_writes:
                add_dep_helper(dr.ins, bw.ins,
                               reason="phase2 reads base after drain")

            # ---------------- Phase 2 ---------------------------------------
            with (
                tc.tile_pool(name="p2", bufs=3) as p2,
                tc.tile_pool(name="p2m", bufs=2) as p2m,
                tc.tile_pool(name="p2s", bufs=1) as p2s,
                tc.tile_pool(name="p2f", bufs=2) as p2f,
                tc.tile_pool(name="p2acc", bufs=1) as p2acc,
                tc.tile_pool(name="p2ps", bufs=2, space="PSUM") as p2ps,
            ):
                def do_batch(ib, nb):
                    nidx = nb * P * K
                    sidx = p2.tile([P, nb * D], I32, tag="sidx")
                    nc.sync.dma_start(
                        sidx[:],
                        bass.AP(src_own, ib * P * D,
                                [[D, P], [P * D, nb], [1, D]]))
                    selv = p2.tile([P, nb * K], F32, tag="selv")
                    for k2 in range(nb):
                        i = ib + k2
                        # ---- per-edge t via window gather ----
                        tgi = p2.tile([P, NTI // 16], I16, tag="tgi")
                        nc.sync.dma_start(
                            tgi[:],
                            tg_idx[:, i * (NTI // 16):(i + 1) * (NTI // 16)])
                        twg = p2m.tile([P, D, 64], F32, tag="twg", bufs=1)
                        g = nc.gpsimd.dma_gather(
                            twg[:], twin[:, :], tgi[:], NTI, NTI, 64,
                            single_packet=False)
                        add_dep_helper(g.ins, dr.ins,
                                       reason="t gather after drain")
                        # q = src & 3; masks m_r = (q == r)
                        q = p2.tile([P, D], I32, tag="q")
                        nc.vector.tensor_scalar(
                            q[:], sidx[:, k2 * D:(k2 + 1) * D], 3, None,
                            op0=mybir.AluOpType.bitwise_and)
                        qm = []
                        for r in range(4):
                            m = p2.tile([P, D], F32, tag=f"qm{r}")
                            nc.vector.tensor_scalar(
                                m[:], q[:], r, None,
                                op0=mybir.AluOpType.is_equal)
                            qm.append(m)
                        tsrc = []
                        for c in range(C):
                            acc = p2.tile([P, D], F32, tag=f"tacc{c}")
                            nc.vector.tensor_tensor(
                                out=acc[:], in0=qm[0][:], in1=twg[:, :, c],
                                op=mybir.AluOpType.mult)
                            for r in range(1, 4):
                                mv = p2.tile([P, D], F32, tag="mv")
                                nc.vector.tensor_tensor(
                                    out=mv[:], in0=qm[r][:],
                                    in1=twg[:, :, 2 * r + c],
                                    op=mybir.AluOpType.mult)
                                nc.vector.tensor_tensor(
                                    out=acc[:], in0=acc[:], in1=mv[:],
                                    op=mybir.AluOpType.add)
                            tsrc.append(acc)

                        # ---- selection (baseline machinery) ----
                        absa = p2.tile([P, D], F32, tag="absa")
                        nc.scalar.activation(
                            absa[:], tsrc[0][:],
                            mybir.ActivationFunctionType.Abs,
                            bias=tneg[:, i * C:i * C + 1])
                        absb = p2.tile([P, D], F32, tag="absb")
                        nc.scalar.activation(
                            absb[:], tsrc[1][:],
                            mybir.ActivationFunctionType.Abs,
                            bias=tneg[:, i * C + 1:i * C + 2])
                        negd = p2.tile([P, D], F32, tag="negd")
                        nc.vector.scalar_tensor_tensor(
                            out=negd[:], in0=absa[:], scalar=-1.0,
                            in1=absb[:],
                            op0=mybir.AluOpType.mult,
                            op1=mybir.AluOpType.subtract)
                        v8a = p2.tile([P, 8], F32, tag="v8a")
                        nc.vector.max(v8a[:], negd[:])
                        negd2 = p2.tile([P, D], F32, tag="negd2")
                        nc.vector.match_replace(
                            out=negd2[:], in_to_replace=v8a[:],
                            in_values=negd[:], imm_value=MINVAL)
                        v8b = p2.tile([P, 8], F32, tag="v8b")
                        nc.vector.max(v8b[:], negd2[:])
                        negd3 = p2.tile([P, D], F32, tag="negd3")
                        nc.vector.match_replace(
                            out=negd3[:], in_to_replace=v8b[:],
                            in_values=negd2[:], imm_value=MINVAL)
                        mask = p2.tile([P, D], F32, tag="mask")
                        nc.vector.tensor_scalar(
                            mask[:], negd3[:], MINVAL, None,
                            op0=mybir.AluOpType.is_equal)
                        srcf = p2.tile([P, D], F32, tag="srcf")
                        nc.vector.tensor_copy(
                            srcf[:], sidx[:, k2 * D:(k2 + 1) * D])
                        msrc = p2.tile([P, D], F32, tag="msrc")
                        nc.vector.scalar_tensor_tensor(
                            out=msrc[:], in0=srcf[:], scalar=1.0,
                            in1=mask[:],
                            op0=mybir.AluOpType.add,
                            op1=mybir.AluOpType.mult)
                        self_f = p2.tile([P, K], F32, tag="self_f")
                        nc.vector.max(self_f[:, 0:8], msrc[:])
                        msrc2 = p2.tile([P, D], F32, tag="msrc2")
                        nc.vector.match_replace(
                            out=msrc2[:], in_to_replace=self_f[:, 0:8],
                            in_values=msrc[:], imm_value=0.0)
                        nc.vector.max(self_f[:, 8:16], msrc2[:])
                        # selv = src+1 of kept edges
                        nc.vector.tensor_copy(
                            selv[:, k2 * K:(k2 + 1) * K], self_f[:])

                    # ---- feature idx prep (batch-wide) ----
                    ps_t = p2ps.tile([K, nb * P], F32, tag="ps_t")
                    for k2 in range(nb):
                        nc.tensor.transpose(
                            ps_t[:, ts(k2, P)],
                            selv[:, k2 * K:(k2 + 1) * K], ident[:])
                    selT = p2m.tile([K, nb * P], F32, tag="selT")
                    nc.scalar.copy(selT[:], ps_t[:])

                    wbufs = []
                    for c in range(NCHUNK):
                        z1 = p2s.tile([K, nb * P], F32, tag="z1")
                        nc.vector.tensor_scalar(
                            z1[:], selT[:], float(c * CH), 0.0,
                            op0=mybir.AluOpType.subtract,
                            op1=mybir.AluOpType.max)
                        m1 = p2s.tile([K, nb * P], F32, tag="m1")
                        nc.vector.tensor_scalar(
                            m1[:], z1[:], float(CH), None,
                            op0=mybir.AluOpType.is_le)
                        iv = p2m.tile([K, nb * P], F32, tag="iv")
                        nc.vector.tensor_tensor(
                            out=iv[:], in0=z1[:], in1=m1[:],
                            op=mybir.AluOpType.mult)
                        ps_w = p2ps.tile([P, nb * P], F32, tag="ps_w")
                        nc.tensor.matmul(out=ps_w[:], lhsT=rep[:],
                                         rhs=iv[:], start=True, stop=True)
                        wb = p2m.tile([P, nb * P], I16, tag="wb", bufs=8)
                        nc.vector.tensor_copy(wb[:], ps_w[:])
                        wbufs.append(wb)

                    # ---- 4 sweep gathers + exact compose ----
                    accb = p2acc.tile([P, nidx], BF16, tag="accb")
                    fb0 = p2f.tile([P, 1, nidx], BF16, tag="fb")
                    nc.gpsimd.dma_gather(
                        fb0[:], fbts[0][:, :],
                        wbufs[0][:], nidx, nidx, F, transpose=True,
                        single_packet=False)
                    fb1 = p2f.tile([P, 1, nidx], BF16, tag="fb")
                    nc.gpsimd.dma_gather(
                        fb1[:], fbts[1][:, :],
                        wbufs[1][:], nidx, nidx, F, transpose=True,
                        single_packet=False)
                    nc.vector.tensor_tensor(
                        out=accb[:],
                        in0=fb0[:].rearrange("p a b -> p (a b)"),
                        in1=fb1[:].rearrange("p a b -> p (a b)"),
                        op=mybir.AluOpType.add)
                    for c in (2, 3):
                        fbc = p2f.tile([P, 1, nidx], BF16, tag="fb")
                        nc.gpsimd.dma_gather(
                            fbc[:], fbts[c][:, :],
                            wbufs[c][:], nidx, nidx, F, transpose=True,
                            single_packet=False)
                        nc.vector.tensor_tensor(
                            out=accb[:], in0=accb[:],
                            in1=fbc[:].rearrange("p a b -> p (a b)"),
                            op=mybir.AluOpType.add)

                    # ---- tree-sum 16 selected columns per dst ----
                    a16 = accb[:].rearrange("p (n k two) -> p n k two", two=2,
                                            k=8)
                    s8 = p2s.tile([P, nidx // 2], BF16, tag="s8")
                    s8v = s8[:].rearrange("p (n k) -> p n k", k=8)
                    nc.vector.tensor_tensor(
                        out=s8v, in0=a16[:, :, :, 0], in1=a16[:, :, :, 1],
                        op=mybir.AluOpType.add)
                    s8p = s8[:].rearrange("p (n k two) -> p n k two", two=2,
                                          k=4)
                    s4 = p2s.tile([P, nidx // 4], BF16, tag="s4")
                    s4v = s4[:].rearrange("p (n k) -> p n k", k=4)
                    nc.vector.tensor_tensor(
                        out=s4v, in0=s8p[:, :, :, 0], in1=s8p[:, :, :, 1],
                        op=mybir.AluOpType.add)
                    s4p = s4[:].rearrange("p (n k two) -> p n k two", two=2,
                                          k=2)
                    s2 = p2s.tile([P, nidx // 8], BF16, tag="s2")
                    s2v = s2[:].rearrange("p (n k) -> p n k", k=2)
                    nc.vector.tensor_tensor(
                        out=s2v, in0=s4p[:, :, :, 0], in1=s4p[:, :, :, 1],
                        op=mybir.AluOpType.add)
                    s2p = s2[:].rearrange("p (n two) -> p n two", two=2)
                    hsum = p2m.tile([P, nb * P], F32, tag="hsum")
                    nc.vector.tensor_tensor(
                        out=hsum[:], in0=s2p[:, :, 0], in1=s2p[:, :, 1],
                        op=mybir.AluOpType.add)
                    bs = p2m.tile([H, nb * P], F32, tag="bs")
                    bsl = nc.sync.dma_start(
                        bs[:], base[:, ib * P:(ib + nb) * P])
                    add_dep_helper(bsl.ins, dr.ins,
                                   reason="base read after drain")
                    # wls is pre-scaled by PKEEP/K on host
                    ps_o = p2ps.tile([H, nb * P], F32, tag="ps_o")
                    nc.tensor.matmul(out=ps_o[:], lhsT=wls[:], rhs=hsum[:],
                                     start=True, stop=True)
                    ob = p2m.tile([H, nb * P], F32, tag="ob")
                    nc.vector.tensor_tensor(
                        out=ob[:], in0=ps_o[:], in1=bs[:],
                        op=mybir.AluOpType.add)
                    nc.sync.dma_start(out_t[:, ib * P:(ib + nb) * P], ob[:])

                for b in range(NBATCH):
                    do_batch(b * TB, TB)
                if TILES > NBATCH * TB:
                    do_batch(NBATCH * TB, TILES - NBATCH * TB)

    nc.compile()
    return nc


_NC_CACHE = {}


def _get_nc():
    key = (NPAD, SHARD, NCORES)
    if key not in _NC_CACHE:
        _NC_CACHE[key] = build()
    return _NC_CACHE[key]


def _wrap16(flat):
    """[n] int array -> [128, n/16] int16 wrapped + replicated."""
    n = flat.size
    w = flat.reshape(n // 16, 16).T.astype(np.int16)
    return np.tile(w, (8, 1))


def make_in_maps(feature, src_ids, W_mlp, b_mlp, W_lin, b_lin):
    fpad = np.zeros((NPAD, F), np.float32)
    fpad[:N] = np.asarray(feature, np.float32)
    spad = np.zeros((NPAD * D,), np.int32)
    spad[:src_ids.size] = np.asarray(src_ids, np.int32).ravel()
    src2d = spad.reshape(NPAD, D)

    fb16 = fpad.astype(ml_dtypes.bfloat16)
    fbts = []
    for c in range(NCHUNK):
        tb = np.zeros((CHROWS, F), ml_dtypes.bfloat16)
        hi = min(CH, NPAD - c * CH)
        if hi > 0:
            tb[1:1 + hi] = fb16[c * CH:c * CH + hi]
        fbts.append(tb)

    rep = np.zeros((K, P), np.float32)
    for z in range(P):
        rep[z % K, z] = 1.0

    wmt = np.ascontiguousarray(np.asarray(W_mlp, np.float32).T)
    wlt = np.ascontiguousarray(np.asarray(W_lin, np.float32).T)
    wls = np.ascontiguousarray(wlt * (PKEEP / K))
    bm = np.asarray(b_mlp, np.float32).reshape(C, 1)
    bl = np.asarray(b_lin, np.float32).reshape(H, 1)

    in_maps = []
    for core in range(NCORES):
        sl = slice(core * SHARD, (core + 1) * SHARD)
        s2 = src2d[sl]
        win = (s2 // 4).astype(np.int16)
        cols = []
        for i in range(TILES):
            blk = win[i * P:(i + 1) * P, :].T   # [D, P]: j = s*128 + p
            cols.append(_wrap16(blk.ravel()))
        tg = np.concatenate(cols, axis=1)
        im = {f"fbt{c}": fbts[c] for c in range(NCHUNK)}
        in_maps.append({
            **im,
            "feat_own": np.ascontiguousarray(fpad[sl]),
            "src_own": np.ascontiguousarray(s2),
            "tg_idx": tg,
            "rep_mat": rep,
            "w_mlp_t": wmt,
            "b_mlp": bm,
            "w_lin_t": wlt,
            "w_lin_s": wls,
            "b_lin": bl,
        })
    return in_maps


def run(feature, src_ids, W_mlp, b_mlp, W_lin, b_lin, **spmd_kwargs):
    nc = _get_nc()
    in_maps = make_in_maps(feature, src_ids, W_mlp, b_mlp, W_lin, b_lin)
    res = run_bass_kernel_spmd(nc, in_maps, core_ids=list(range(NCORES)),
                               **spmd_kwargs)
    outs = [res.results[c]["out_t"] for c in range(NCORES)]
    full = np.concatenate([o.T for o in outs], axis=0)[:N]
    return np.ascontiguousarray(full, dtype=np.float32), res


def kernel(feature, src_ids, W_mlp, b_mlp, W_lin, b_lin):
    out, _ = run(feature, src_ids, W_mlp, b_mlp, W_lin, b_lin)
    return out
